# revision 23
# baseline (speedup 1.0000x reference)
"""AdaConv2d (per-sample masked 3x3 conv) on 8 TRN2 NeuronCores.

Strategy (data-parallel, per sharding hint):
  - 64 samples sharded 8-per-core; kernel_base/kernel_mask replicated.
  - Two samples share one 128-partition SBUF tile: sample A's padded
    image (one input channel per partition) in partitions 0-63, sample
    B's in 64-127. No shifted copies -> input DMA is 1x the image.
  - Each of the 9 conv taps is one K=64 matmul; per (tap, 4-row block)
    four M=64 matmuls run concurrently on the four 64x64 quadrants of
    the PE array (tile_position auto-derived from base partitions):
      (row 0,  col 0)  = sample A, even block -> psA[0:64]
      (row 0,  col 64) = sample A, odd block  -> psA[64:128]
      (row 64, col 0)  = sample B, even block -> psB[0:64]
      (row 64, col 64) = sample B, odd block  -> psB[64:128]
    so all 16384 MACs/cycle are live on every pass (100% MAC
    utilization; the PE roofline for this decomposition is ~94us).
  - Per-sample kernels (kernel_base * kernel_mask[label], bf16, lhsT
    layout) are precomputed on the host: they are tiny (~300KB/pair)
    and shipping them ready removes the on-device mask-multiply from
    the critical path at kernel start.
  - Output staged bf16 in SBUF, one dma_start per (pair, round) into a
    raw-layout DRAM tensor (dma_start issue cost is ~600ns fixed on
    the sequencer); the host transposes back to [B, OC, H, W].
"""
import numpy as np
import ml_dtypes

import concourse.bass as bass  # noqa: F401  (registers engines)
import concourse.tile as tile
from concourse import bacc, mybir
from concourse.bass_utils import run_bass_kernel_spmd

NCORES = 8
SPC = 8            # samples per core
PAIRS = SPC // 2   # two samples share one 128-partition tile
H = W = 112
IC = OC = 64
ND = 4             # demographic groups
PW = H + 2         # padded width/height
PHW = PW * PW
RB = 4             # output rows per matmul block
N = RB * W         # 448 columns per matmul (one PSUM bank)
ROUNDS = H // (2 * RB)   # 14 rounds of (even, odd) blocks per sample
NTAP = 9
FUSE_EPOCH = 9
F32 = mybir.dt.float32
BF16 = mybir.dt.bfloat16

# x chunk boundaries (padded-row units): fine enough that the first
# rounds' matmuls only wait on the first chunk or two; pair 0 gets
# extra-fine leading chunks so its first rounds start ASAP
XROWS = [0, 16, 32, 48, 64, 80, 96, PW]
XROWS0 = [0, 8, 16, 32, 48, 64, 80, 96, PW]
NWARM = 32         # dummy matmuls to lift the PE HAM clock-gate early

_CACHE = {}


def _build():
    nc = bacc.Bacc("TRN2", target_bir_lowering=False, debug=False,
                   num_devices=NCORES)
    xs = nc.dram_tensor("xs", [PAIRS, 128, PHW], BF16,
                        kind="ExternalInput").ap()
    wd = nc.dram_tensor("wd", [PAIRS, 128, NTAP * 128], BF16,
                        kind="ExternalInput").ap()
    out = nc.dram_tensor("out", [PAIRS, ROUNDS, 2, 2, OC, N], BF16,
                         kind="ExternalOutput").ap()

    # [pair, round, blk*oc (partition), sample-in-pair, rb*w]
    ov = out.rearrange("pr r b k oc f -> pr r (k oc) b f")

    with tile.TileContext(nc) as tc:
        with (
            tc.tile_pool(name="xp", bufs=3) as xp,
            tc.tile_pool(name="wp", bufs=2) as wp,
            tc.tile_pool(name="stage", bufs=8) as stp,
            tc.tile_pool(name="psum", bufs=3, space="PSUM") as pp,
            tc.tile_pool(name="warm", bufs=1) as wmp,
            tc.tile_pool(name="warmps", bufs=1, space="PSUM") as wpp,
        ):
            # warm up the PE HAM clock-gate (1.2 -> 2.4 GHz needs ~3.4us
            # of sustained activity) with dummy matmuls on scratch data
            # while the first pair's inputs are still in flight
            warm = wmp.tile([128, 128], BF16, name="warm", tag="warm")
            nc.gpsimd.memset(warm[:], 0)
            psW = wpp.tile([64, 128], F32, name="psW", tag="psW")
            for _ in range(NWARM):
                nc.tensor.matmul(psW[:], warm[:, 0:64], warm[:],
                                 start=True, stop=True)

            for pr in range(PAIRS):
                wt = wp.tile([128, NTAP * 128], BF16, name="wt", tag="wt")
                # pair 0: sync queue is idle, issue there for earliest
                # landing; later pairs: sync is busy with output DMAs, so
                # scalar (whose x-chunk issues are long done) is sooner
                weng = nc.sync if pr == 0 else nc.scalar
                weng.dma_start(wt[:], wd[pr])
                w3 = wt.rearrange("p (j m) -> p j m", m=128)

                xt = xp.tile([128, PHW], BF16, name="xt", tag="xt")
                x3 = xt.rearrange("p (r c) -> p r c", c=PW)
                xrows = XROWS0 if pr == 0 else XROWS
                for q in range(len(xrows) - 1):
                    qs, qe = xrows[q] * PW, xrows[q + 1] * PW
                    nc.scalar.dma_start(xt[:, qs:qe], xs[pr][:, qs:qe])

                for rnd in range(ROUNDS):
                    psA = pp.tile([128, N], F32, name="psA", tag="psA")
                    psB = pp.tile([128, N], F32, name="psB", tag="psB")
                    for j in range(NTAP):
                        dy, dx = divmod(j, 3)
                        first, last = (j == 0), (j == NTAP - 1)
                        for blk in range(2):
                            r0 = rnd * 2 * RB + blk * RB + dy
                            pc = blk * 64
                            rA = x3[0:64, r0:r0 + RB, dx:dx + W]
                            rB = x3[64:128, r0:r0 + RB, dx:dx + W]
                            nc.tensor.matmul(psA[pc:pc + 64, :],
                                             w3[0:64, j, pc:pc + 64], rA,
                                             start=first, stop=last)
                            nc.tensor.matmul(psB[pc:pc + 64, :],
                                             w3[64:128, j, pc:pc + 64], rB,
                                             start=first, stop=last)

                    st = stp.tile([128, 2, N], BF16, name="st", tag="st")
                    nc.vector.tensor_copy(st[:, 0, :], psA[:])
                    nc.vector.tensor_copy(st[:, 1, :], psB[:])
                    nc.sync.dma_start(ov[pr, rnd], st[:])

    nc.compile()
    return nc


def get_nc():
    if "nc" not in _CACHE:
        _CACHE["nc"] = _build()
    return _CACHE["nc"]


def make_in_maps(x, kernel_base, kernel_mask, demog_label, epoch):
    kb = np.asarray(kernel_base, dtype=np.float32)
    km = np.asarray(kernel_mask, dtype=np.float32)
    labels = np.asarray(demog_label).astype(np.int64)
    if int(np.asarray(epoch)) >= FUSE_EPOCH:
        labels = np.zeros_like(labels)

    B = labels.shape[0]
    # padded bf16 image per sample (layout only); pairs share a tile
    xb = np.asarray(x, dtype=np.float32).astype(ml_dtypes.bfloat16)
    xpad = np.zeros((B, IC, PW, PW), dtype=ml_dtypes.bfloat16)
    xpad[:, :, 1:H + 1, 1:W + 1] = xb
    xfull = xpad.reshape(B // 2, 128, PHW)

    # per-sample lhsT weights [ic, tap, oc], duplicated across the two
    # 64-col halves of the PE array
    kbT = kb.reshape(OC, IC, NTAP).transpose(1, 2, 0)   # [ic, j, oc]
    km9 = km.reshape(ND, IC, NTAP)                      # [d, ic, j]
    # ws[d, ic, j, oc] = kb[oc, ic, j] * km[d, ic, j]
    ws = kbT[None] * km9[:, :, :, None]                 # [d, ic, j, oc]
    wdup = np.concatenate([ws, ws], axis=3)             # [d, ic, j, 128]
    wdup = wdup.reshape(ND, IC, NTAP * 128).astype(ml_dtypes.bfloat16)

    in_maps = []
    for c in range(NCORES):
        lab = labels[c * SPC:(c + 1) * SPC]
        wdc = np.zeros((PAIRS, 128, NTAP * 128), dtype=ml_dtypes.bfloat16)
        for p in range(PAIRS):
            wdc[p, 0:IC] = wdup[lab[2 * p]]
            wdc[p, IC:] = wdup[lab[2 * p + 1]]
        in_maps.append({
            "xs": np.ascontiguousarray(
                xfull[c * PAIRS:(c + 1) * PAIRS]),
            "wd": wdc,
        })
    return in_maps


def kernel(x, kernel_base, kernel_mask, demog_label, epoch):
    nc = get_nc()
    in_maps = make_in_maps(x, kernel_base, kernel_mask, demog_label, epoch)
    res = run_bass_kernel_spmd(nc, in_maps, list(range(NCORES)))
    outs = []
    for c in range(NCORES):
        raw = res.results[c]["out"].astype(np.float32)
        # [PAIRS, ROUNDS, b, blk, OC, RB, W] -> [PAIRS, b, OC, R, blk, RB, W]
        raw = raw.reshape(PAIRS, ROUNDS, 2, 2, OC, RB, W)
        raw = raw.transpose(0, 2, 4, 1, 3, 5, 6)
        outs.append(raw.reshape(SPC, OC, H, W))
    return np.concatenate(outs, axis=0)


# revision 24
# speedup vs baseline: 1.0491x; 1.0491x over previous
"""AdaConv2d (per-sample masked 3x3 conv) on 8 TRN2 NeuronCores.

Strategy (data-parallel, per sharding hint):
  - 64 samples sharded 8-per-core; kernel_base/kernel_mask replicated.
  - Two samples share one 128-partition SBUF tile: sample A's padded
    image (one input channel per partition) in partitions 0-63, sample
    B's in 64-127. No shifted copies -> input DMA is 1x the image.
  - Each of the 9 conv taps is one K=64 matmul; per (tap, 4-row block)
    four M=64 matmuls run concurrently on the four 64x64 quadrants of
    the PE array (tile_position auto-derived from base partitions):
      (row 0,  col 0)  = sample A, even block -> psA[0:64]
      (row 0,  col 64) = sample A, odd block  -> psA[64:128]
      (row 64, col 0)  = sample B, even block -> psB[0:64]
      (row 64, col 64) = sample B, odd block  -> psB[64:128]
    so all 16384 MACs/cycle are live on every pass (100% MAC
    utilization; the PE roofline for this decomposition is ~94us).
  - Per-sample kernels (kernel_base * kernel_mask[label], bf16, lhsT
    layout) are precomputed on the host: they are tiny (~300KB/pair)
    and shipping them ready removes the on-device mask-multiply from
    the critical path at kernel start.
  - Output staged bf16 in SBUF, one dma_start per (pair, round) into a
    raw-layout DRAM tensor (dma_start issue cost is ~600ns fixed on
    the sequencer); the host transposes back to [B, OC, H, W].
"""
import numpy as np
import ml_dtypes

import concourse.bass as bass  # noqa: F401  (registers engines)
import concourse.tile as tile
from concourse import bacc, mybir
from concourse.bass_utils import run_bass_kernel_spmd

NCORES = 8
SPC = 8            # samples per core
PAIRS = SPC // 2   # two samples share one 128-partition tile
H = W = 112
IC = OC = 64
ND = 4             # demographic groups
PW = H + 2         # padded width/height
PHW = PW * PW
RB = 4             # output rows per matmul block
N = RB * W         # 448 columns per matmul (one PSUM bank)
ROUNDS = H // (2 * RB)   # 14 rounds of (even, odd) blocks per sample
NTAP = 9
FUSE_EPOCH = 9
F32 = mybir.dt.float32
BF16 = mybir.dt.bfloat16

# x chunk boundaries (padded-row units): fine enough that the first
# rounds' matmuls only wait on the first chunk or two; pair 0 gets
# extra-fine leading chunks so its first rounds start ASAP
XROWS = [0, 16, 32, 48, 64, 80, 96, PW]
XROWS0 = [0, 8, 16, 32, 48, 64, 80, 96, PW]
NWARM = 32         # dummy matmuls to lift the PE HAM clock-gate early

_CACHE = {}


def _build():
    nc = bacc.Bacc("TRN2", target_bir_lowering=False, debug=False,
                   num_devices=NCORES)
    xs = nc.dram_tensor("xs", [PAIRS, 128, PHW], BF16,
                        kind="ExternalInput").ap()
    wd = nc.dram_tensor("wd", [PAIRS, 128, NTAP * 128], BF16,
                        kind="ExternalInput").ap()
    out = nc.dram_tensor("out", [PAIRS, ROUNDS, 2, 2, OC, N], BF16,
                         kind="ExternalOutput").ap()

    # [pair, round, blk*oc (partition), sample-in-pair, rb*w]
    ov = out.rearrange("pr r b k oc f -> pr r (k oc) b f")

    with tile.TileContext(nc) as tc:
        with (
            tc.tile_pool(name="xp", bufs=3) as xp,
            tc.tile_pool(name="wp", bufs=2) as wp,
            tc.tile_pool(name="stage", bufs=12) as stp,
            tc.tile_pool(name="psum", bufs=3, space="PSUM") as pp,
            tc.tile_pool(name="warm", bufs=1) as wmp,
            tc.tile_pool(name="warmps", bufs=1, space="PSUM") as wpp,
        ):
            # warm up the PE HAM clock-gate (1.2 -> 2.4 GHz needs ~3.4us
            # of sustained activity) with dummy matmuls on scratch data
            # while the first pair's inputs are still in flight
            warm = wmp.tile([128, 128], BF16, name="warm", tag="warm")
            nc.gpsimd.memset(warm[:], 0)
            psW = wpp.tile([64, 128], F32, name="psW", tag="psW")
            for _ in range(NWARM):
                nc.tensor.matmul(psW[:], warm[:, 0:64], warm[:],
                                 start=True, stop=True)

            for pr in range(PAIRS):
                wt = wp.tile([128, NTAP * 128], BF16, name="wt", tag="wt")
                # pair 0: sync queue is idle, issue there for earliest
                # landing; later pairs: sync is busy with output DMAs, so
                # scalar (whose x-chunk issues are long done) is sooner
                weng = nc.sync if pr == 0 else nc.scalar
                weng.dma_start(wt[:], wd[pr])
                w3 = wt.rearrange("p (j m) -> p j m", m=128)

                xt = xp.tile([128, PHW], BF16, name="xt", tag="xt")
                x3 = xt.rearrange("p (r c) -> p r c", c=PW)
                xrows = XROWS0 if pr == 0 else XROWS
                for q in range(len(xrows) - 1):
                    qs, qe = xrows[q] * PW, xrows[q + 1] * PW
                    nc.scalar.dma_start(xt[:, qs:qe], xs[pr][:, qs:qe])

                for rnd in range(ROUNDS):
                    psA = pp.tile([128, N], F32, name="psA", tag="psA")
                    psB = pp.tile([128, N], F32, name="psB", tag="psB")
                    for j in range(NTAP):
                        dy, dx = divmod(j, 3)
                        first, last = (j == 0), (j == NTAP - 1)
                        for blk in range(2):
                            r0 = rnd * 2 * RB + blk * RB + dy
                            pc = blk * 64
                            rA = x3[0:64, r0:r0 + RB, dx:dx + W]
                            rB = x3[64:128, r0:r0 + RB, dx:dx + W]
                            nc.tensor.matmul(psA[pc:pc + 64, :],
                                             w3[0:64, j, pc:pc + 64], rA,
                                             start=first, stop=last)
                            nc.tensor.matmul(psB[pc:pc + 64, :],
                                             w3[64:128, j, pc:pc + 64], rB,
                                             start=first, stop=last)

                    st = stp.tile([128, 2, N], BF16, name="st", tag="st")
                    nc.vector.tensor_copy(st[:, 0, :], psA[:])
                    nc.vector.tensor_copy(st[:, 1, :], psB[:])
                    nc.sync.dma_start(ov[pr, rnd], st[:])

    nc.compile()
    return nc


def get_nc():
    if "nc" not in _CACHE:
        _CACHE["nc"] = _build()
    return _CACHE["nc"]


def make_in_maps(x, kernel_base, kernel_mask, demog_label, epoch):
    kb = np.asarray(kernel_base, dtype=np.float32)
    km = np.asarray(kernel_mask, dtype=np.float32)
    labels = np.asarray(demog_label).astype(np.int64)
    if int(np.asarray(epoch)) >= FUSE_EPOCH:
        labels = np.zeros_like(labels)

    B = labels.shape[0]
    # padded bf16 image per sample (layout only); pairs share a tile
    xb = np.asarray(x, dtype=np.float32).astype(ml_dtypes.bfloat16)
    xpad = np.zeros((B, IC, PW, PW), dtype=ml_dtypes.bfloat16)
    xpad[:, :, 1:H + 1, 1:W + 1] = xb
    xfull = xpad.reshape(B // 2, 128, PHW)

    # per-sample lhsT weights [ic, tap, oc], duplicated across the two
    # 64-col halves of the PE array
    kbT = kb.reshape(OC, IC, NTAP).transpose(1, 2, 0)   # [ic, j, oc]
    km9 = km.reshape(ND, IC, NTAP)                      # [d, ic, j]
    # ws[d, ic, j, oc] = kb[oc, ic, j] * km[d, ic, j]
    ws = kbT[None] * km9[:, :, :, None]                 # [d, ic, j, oc]
    wdup = np.concatenate([ws, ws], axis=3)             # [d, ic, j, 128]
    wdup = wdup.reshape(ND, IC, NTAP * 128).astype(ml_dtypes.bfloat16)

    in_maps = []
    for c in range(NCORES):
        lab = labels[c * SPC:(c + 1) * SPC]
        wdc = np.zeros((PAIRS, 128, NTAP * 128), dtype=ml_dtypes.bfloat16)
        for p in range(PAIRS):
            wdc[p, 0:IC] = wdup[lab[2 * p]]
            wdc[p, IC:] = wdup[lab[2 * p + 1]]
        in_maps.append({
            "xs": np.ascontiguousarray(
                xfull[c * PAIRS:(c + 1) * PAIRS]),
            "wd": wdc,
        })
    return in_maps


def kernel(x, kernel_base, kernel_mask, demog_label, epoch):
    nc = get_nc()
    in_maps = make_in_maps(x, kernel_base, kernel_mask, demog_label, epoch)
    res = run_bass_kernel_spmd(nc, in_maps, list(range(NCORES)))
    outs = []
    for c in range(NCORES):
        raw = res.results[c]["out"].astype(np.float32)
        # [PAIRS, ROUNDS, b, blk, OC, RB, W] -> [PAIRS, b, OC, R, blk, RB, W]
        raw = raw.reshape(PAIRS, ROUNDS, 2, 2, OC, RB, W)
        raw = raw.transpose(0, 2, 4, 1, 3, 5, 6)
        outs.append(raw.reshape(SPC, OC, H, W))
    return np.concatenate(outs, axis=0)
